# revision 39
# baseline (speedup 1.0000x reference)
"""Trainium2 Bass kernel for nn_LlamaApproximatedAttention.

Math (per batch b, with hs = hidden_states[b] [S, H]):
    F_h = W_seq @ hs            # [R, H]   (contract s)
    F_s = hs @ W_hid.T          # [S, R]   (contract h)
    out = F_s @ F_h             # [S, H]   (contract r)

Sharding: 8 cores = (batch b = c//2, seq-half j = c%2). Each core receives the
full hs[b] with its own half's rows first (host-side roll), computes F_h fully,
and F_s / out only for its own half. Pure SPMD.

DMA layout: hs is pre-packed on the host into 4 groups of 4 s-tiles,
[g][p][tile*H] fp16, so each group load is a single 16KB-contiguous descriptor
per partition (128 descriptors total). The output uses the same packing
([ip][p][w*H], 8KB/partition/store). Loads go on the SP DGE queue, stores on
the Pool (SWDGE) queue so stores never block next-iteration load triggers.

SBUF pools are opened OUTSIDE the timing loop and the loop body is emitted
twice per hardware For iteration, so tile-pool buffer rotation carries across
iterations (cross-iteration load prefetch / store draining). Constants load
once, outside the loop.

All PE inputs are float16 (PSUM accumulation stays fp32): halves DMA traffic
(the bottleneck) vs f32, and PE transposes run 1.0 cycles/row vs 1.5 for f32r.
grid_chw is unused by the math (it enumerates the full (s, h) grid).
"""

import numpy as np

import concourse.bass as bass  # noqa: F401  (engine namespaces hang off nc)
import concourse.mybir as mybir
import concourse.tile as tile
from concourse import bacc
from concourse.bass_utils import run_bass_kernel_spmd

B, S, H, R = 4, 2048, 2048, 64
N_CORES = 8
P = 128
T = S // P            # 16 s-tiles (also 16 h-tiles)
OWN_T = T // 2        # 8 own s-tiles per core
CHUNK = 512
NCH = H // CHUNK      # 4 h-chunks
NG = 4                # s-tile groups of 4 (load granularity)

f16 = mybir.dt.float16
f32 = mybir.dt.float32

# Pair-wise AllReduce of the partial F_h: each core loads only its own seq
# half of hs (halves load DMA + einsum1) and the (b, j=0)/(b, j=1) pair
# reduces F_h over a device-to-device collective.
COLLECTIVE = False
CC_GROUPS = [[0, 1], [2, 3], [4, 5], [6, 7]]


def build_nc(reps: int = 1, mode: str = "full", unroll: bool = False):
    """Build + bacc-compile the SPMD kernel. reps>1 wraps two copies of the
    body in a hardware For loop of reps//2 (reps must be even), so tile-pool
    rotation double-buffers across iterations. Output is idempotent.

    mode (timing diagnostics, progressively enables phases):
      "dma"  : input loads + output stores only
      "e1"   : + einsum1 matmuls
      "tr"   : + PE transposes
      "e2"   : + einsum2 matmuls/copies
      "full" : + einsum3 (the real kernel)
    """
    lvl = {"dma": 0, "e1": 1, "tr": 2, "e2": 3, "full": 4}[mode]
    nc = bacc.Bacc(
        "TRN2",
        target_bir_lowering=False,
        debug=False,
        enable_asserts=True,
        num_devices=N_CORES,
    )

    # group g holds s-tiles 4g..4g+3 (of the rows this core receives);
    # per-partition line is 4*H fp16 = 16KB contiguous
    ngl = 2 if COLLECTIVE else NG       # groups loaded per core
    n_t = 4 * ngl                       # s-tiles contracted by einsum1 here
    hsg = nc.dram_tensor("hsg", [ngl, P, 4 * H], f16, kind="ExternalInput").ap()
    wst = nc.dram_tensor("wst", [P, n_t * R], f16, kind="ExternalInput").ap()
    wht = nc.dram_tensor("wht", [P, T * R], f16, kind="ExternalInput").ap()
    ident = nc.dram_tensor("ident", [P, P], f16, kind="ExternalInput").ap()
    # pair ip holds s-tiles (2ip, 2ip+1); 2*H fp16 = 8KB/partition contiguous
    out = nc.dram_tensor("out", [NG, P, 2 * H], f16, kind="ExternalOutput").ap()

    with tile.TileContext(nc) as tc:
        with (
            tc.tile_pool(name="const", bufs=1) as cpool,
            tc.tile_pool(name="raw", bufs=2 * ngl) as rpool,
            tc.tile_pool(name="hsT", bufs=8) as hpool,
            tc.tile_pool(name="fact", bufs=2) as fpool,
            tc.tile_pool(name="outsb", bufs=4) as opool,
            tc.tile_pool(name="ccb", bufs=2, space="DRAM") as ccpool,
        ):
            # constants: loaded once, outside the timing loop
            wst_t = cpool.tile([P, n_t * R], f16, tag="wst")
            wht_t = cpool.tile([P, T * R], f16, tag="wht")
            id_t = cpool.tile([P, P], f16, tag="id")
            nc.gpsimd.dma_start(wst_t[:], wst)
            nc.gpsimd.dma_start(wht_t[:], wht)
            nc.gpsimd.dma_start(id_t[:], ident)
            fake_out = None
            if lvl < 4:
                fake_out = cpool.tile([P, 2 * H], f16, tag="fake_out")
                nc.gpsimd.memset(fake_out[:], 0.0)

            def body(_=None):
                # per-group F_s.T tiles so einsum3's ip0/ip1 depend only on
                # group 0's copies (a single tile would serialize on group 1)
                fst_g = [
                    fpool.tile([P, 4 * P], f16, tag=f"fst{g}", name=f"fst{g}")
                    for g in range(2)
                ]
                fh_dup = fpool.tile([P, H], f16, tag="fh")            # [128, 2048]

                raws = []
                for g in range(ngl):
                    raw = rpool.tile([P, 4 * H], f16, tag="raw")
                    nc.sync.dma_start(raw[:], hsg[g])
                    raws.append(raw)

                with tc.tile_pool(name="ps_fh", bufs=1, space="PSUM") as fh_ps_pool:
                    ps_fh = fh_ps_pool.tile([R, H], f32, tag="fh")     # [64, 2048]

                    def e1(g):
                        # einsum1: accumulate partial F_h chunks from group g
                        raw = raws[g]
                        for q in range(4 if lvl >= 1 else 0):
                            t = 4 * g + q
                            for c in range(NCH):
                                nc.tensor.matmul(
                                    ps_fh[:, c * CHUNK:(c + 1) * CHUNK],
                                    wst_t[:, t * R:(t + 1) * R],
                                    raw[:, q * H + c * CHUNK:q * H + (c + 1) * CHUNK],
                                    start=(t == 0),
                                    stop=(t == n_t - 1),
                                )

                    def tr_e2(g, tr_ps, fs_ps):
                        # einsum2 for own group g (s-tiles 4g..4g+3).
                        # einsum2's matmul for h-tile k is emitted after the
                        # transposes for k+1 (software pipelining), so the PE
                        # never stalls on the PSUM->SBUF copy of k.
                        raw = raws[g]
                        ps_fs_g = None
                        if lvl >= 3:
                            ps_fs_g = fs_ps.tile([R, CHUNK], f32, tag="fs")
                        hsTs = {}
                        for k in range(T + 1):  # h-tiles (+1 drain)
                            if k < T:
                                ps_tr = tr_ps.tile([P, 4 * P], f16, tag="tr")
                                for q in range(4):  # s-tiles 4g+q
                                    src = raw[:, q * H + k * P:q * H + (k + 1) * P]
                                    nc.tensor.matmul(
                                        ps_tr[:, q * P:(q + 1) * P],
                                        src,
                                        id_t[:],
                                        is_transpose=True,
                                        start=(q == 0),
                                        stop=(q == 3),
                                    )
                                hsT = hpool.tile([P, 4 * P], f16, tag="hsT")
                                if k % 3 == 1:
                                    nc.scalar.copy(hsT[:], ps_tr[:])
                                else:
                                    nc.vector.tensor_copy(hsT[:], ps_tr[:])
                                hsTs[k] = hsT
                            if lvl >= 3 and k > 0:
                                nc.tensor.matmul(
                                    ps_fs_g[:],
                                    wht_t[:, (k - 1) * R:k * R],
                                    hsTs.pop(k - 1),
                                    start=(k - 1 == 0),
                                    stop=(k - 1 == T - 1),
                                )
                        if lvl >= 3:
                            # F_s.T chunk -> SBUF, duplicated to both halves
                            nc.vector.tensor_copy(fst_g[g][0:R, :], ps_fs_g[:])
                            nc.scalar.copy(fst_g[g][R:2 * R, :], ps_fs_g[:])

                    def fh_copies():
                        # F_h -> SBUF, duplicated to both partition halves.
                        # Emitted before tr_e2(1): the copies run on DVE/Act
                        # while the PE does group 1's transposes, so einsum3
                        # starts with no PE idle (keeps the PE p-state high).
                        if lvl < 4:
                            return
                        for c in range(NCH):
                            sl = slice(c * CHUNK, (c + 1) * CHUNK)
                            if c % 2 == 0:
                                nc.vector.tensor_copy(fh_dup[0:R, sl], ps_fh[:, sl])
                                nc.scalar.copy(fh_dup[R:2 * R, sl], ps_fh[:, sl])
                            else:
                                nc.scalar.copy(fh_dup[0:R, sl], ps_fh[:, sl])
                                nc.vector.tensor_copy(fh_dup[R:2 * R, sl], ps_fh[:, sl])

                    def fh_allreduce():
                        # partial F_h -> fp16 SBUF -> DRAM bounce -> pairwise
                        # AllReduce -> back into both halves of fh_dup. The
                        # whole chain overlaps the PE's transpose phase.
                        if lvl < 4:
                            return
                        fhp = fpool.tile([R, H], f16, tag="fhp")
                        for c in range(NCH):
                            sl = slice(c * CHUNK, (c + 1) * CHUNK)
                            if c % 2 == 0:
                                nc.vector.tensor_copy(fhp[:, sl], ps_fh[:, sl])
                            else:
                                nc.scalar.copy(fhp[:, sl], ps_fh[:, sl])
                        in_b = ccpool.tile([R, H], f16, tag="cc_in")
                        out_b = ccpool.tile([R, H], f16, tag="cc_out")
                        nc.scalar.dma_start(in_b[:], fhp[:])
                        nc.gpsimd.collective_compute(
                            "AllReduce",
                            mybir.AluOpType.add,
                            replica_groups=CC_GROUPS,
                            ins=[in_b.opt()],
                            outs=[out_b.opt()],
                        )
                        nc.gpsimd.dma_start(fh_dup[0:R, :], out_b[:])
                        nc.gpsimd.dma_start(fh_dup[R:2 * R, :], out_b[:])

                    with (
                        tc.tile_pool(name="ps_tr", bufs=2, space="PSUM") as tr_ps,
                        tc.tile_pool(name="ps_fs", bufs=2, space="PSUM") as fs_ps,
                    ):
                        if COLLECTIVE:
                            e1(0)
                            e1(1)
                            fh_allreduce()
                            if lvl >= 2:
                                tr_e2(0, tr_ps, fs_ps)
                                tr_e2(1, tr_ps, fs_ps)
                        else:
                            e1(0)
                            if lvl >= 2:
                                tr_e2(0, tr_ps, fs_ps)
                            e1(1)
                            e1(2)
                            e1(3)
                            fh_copies()
                            if lvl >= 2:
                                tr_e2(1, tr_ps, fs_ps)

                # einsum3 (K=64, row-packed pairs) + output stores.
                # PSUM granularity [128, 512] with 8 bufs: fine-grained drain
                # so the PE never backs up on the PSUM->SBUF cast copies.
                with tc.tile_pool(name="ps_o", bufs=8, space="PSUM") as po:
                    for ip in range(OWN_T // 2):
                        if lvl < 4:
                            nc.gpsimd.dma_start(out[ip], fake_out[:])
                            continue
                        outsb = opool.tile([P, 2 * H], f16, tag="outsb")
                        for u in range(8):  # (hf, c) units
                            hf, c = u % 2, u // 2
                            i = 2 * ip + hf
                            base = R * hf
                            ps_o = po.tile([P, CHUNK], f32, tag="o")
                            nc.tensor.matmul(
                                ps_o[:],
                                fst_g[i // 4][base:base + R, (i % 4) * P:(i % 4 + 1) * P],
                                fh_dup[base:base + R, c * CHUNK:(c + 1) * CHUNK],
                                start=True,
                                stop=True,
                            )
                            dst = outsb[:, hf * H + c * CHUNK:(hf * H + (c + 1) * CHUNK)]
                            if u % 2 == 0:
                                nc.vector.tensor_copy(dst, ps_o[:])
                            else:
                                nc.scalar.copy(dst, ps_o[:])
                        nc.gpsimd.dma_start(out[ip], outsb[:])

            if unroll:
                for _ in range(reps):
                    body()
            elif reps == 1:
                body()
            else:
                nbody = 4 if reps % 4 == 0 else 2
                assert reps % nbody == 0, "reps must be even (2 bodies per For_i)"
                with tc.For_i(0, reps // nbody, 1):
                    for _ in range(nbody):
                        body()

    nc.compile()
    return nc


def _tile_weight(w_t: np.ndarray) -> np.ndarray:
    """[2048, 64] -> [128, 16*64] stack where tile t = cols [64t:64t+64]."""
    return np.ascontiguousarray(
        w_t.reshape(T, P, R).transpose(1, 0, 2).reshape(P, T * R)
    ).astype(np.float16)


def _pack_hs(hs_c: np.ndarray) -> np.ndarray:
    """[rows, 2048] -> [rows/512, 128, 4*H]: group g = s-tiles 4g..4g+3, row p
    of group g holds tiles' rows s = g*512 + q*128 + p concatenated over q."""
    ng = hs_c.shape[0] // 512
    return np.ascontiguousarray(
        hs_c.reshape(ng, 4, P, H).transpose(0, 2, 1, 3).reshape(ng, P, 4 * H)
    )


def _unpack_out(o: np.ndarray) -> np.ndarray:
    """[NG, 128, 2*H] -> [1024, 2048]: s = ip*256 + w*128 + p."""
    return np.ascontiguousarray(
        o.reshape(NG, P, 2, H).transpose(0, 2, 1, 3).reshape(S // 2, H)
    )


def _tile_weight_half(w_t: np.ndarray) -> np.ndarray:
    """[1024, 64] -> [128, 8*64] stack where tile t = cols [64t:64t+64]."""
    return np.ascontiguousarray(
        w_t.reshape(T // 2, P, R).transpose(1, 0, 2).reshape(P, T // 2 * R)
    ).astype(np.float16)


def build_in_maps(hs_all: np.ndarray, w_seq: np.ndarray, w_hid: np.ndarray):
    ident = np.eye(P, dtype=np.float16)
    wht_tiled = _tile_weight(np.ascontiguousarray(w_hid.T))
    hs_f16 = hs_all.astype(np.float16)
    if COLLECTIVE:
        wst_halves = {
            j: _tile_weight_half(
                np.ascontiguousarray(w_seq.T[j * (S // 2):(j + 1) * (S // 2)])
            )
            for j in range(2)
        }
    else:
        wst_halves = {
            j: _tile_weight(np.roll(w_seq.T, -(S // 2) * j, axis=0))
            for j in range(2)
        }
    in_maps = []
    for c in range(N_CORES):
        b, j = c // 2, c % 2
        hsb = hs_f16[b]
        if COLLECTIVE:
            hs_c = hsb[j * (S // 2):(j + 1) * (S // 2)]
        else:
            hs_c = hsb if j == 0 else np.roll(hsb, -(S // 2), axis=0)
        in_maps.append(
            {"hsg": _pack_hs(hs_c), "wst": wst_halves[j], "wht": wht_tiled,
             "ident": ident}
        )
    return in_maps


_NC_CACHE: dict = {}


def kernel(**inputs) -> np.ndarray:
    hs_all = np.asarray(inputs["hidden_states"], dtype=np.float32)
    w_seq = np.asarray(inputs["W_seq"], dtype=np.float32)
    w_hid = np.asarray(inputs["W_hid"], dtype=np.float32)

    if "nc" not in _NC_CACHE:
        _NC_CACHE["nc"] = build_nc(1)
    nc = _NC_CACHE["nc"]

    in_maps = build_in_maps(hs_all, w_seq, w_hid)
    res = run_bass_kernel_spmd(nc, in_maps, core_ids=list(range(N_CORES)))

    out_full = np.empty((B, S, H), dtype=np.float32)
    for c in range(N_CORES):
        b, j = c // 2, c % 2
        out_full[b, j * (S // 2):(j + 1) * (S // 2), :] = _unpack_out(
            res.results[c]["out"]
        )
    return out_full


# revision 45
# speedup vs baseline: 1.2231x; 1.2231x over previous
"""Trainium2 Bass kernel for nn_LlamaApproximatedAttention.

Math (per batch b, with hs = hidden_states[b] [S, H]):
    F_h = W_seq @ hs            # [R, H]   (contract s)
    F_s = hs @ W_hid.T          # [S, R]   (contract h)
    out = F_s @ F_h             # [S, H]   (contract r)

Sharding: 8 cores = (batch b = c//2, seq-half j = c%2). Each core receives the
full hs[b] with its own half's rows first (host-side roll), computes F_h fully,
and F_s / out only for its own half. Pure SPMD.

DMA layout: hs is pre-packed on the host into 4 groups of 4 s-tiles,
[g][p][tile*H] fp16, so each group load is a single 16KB-contiguous descriptor
per partition (128 descriptors total). The output uses the same packing
([ip][p][w*H], 8KB/partition/store). Loads go on the SP DGE queue, stores on
the Pool (SWDGE) queue so stores never block next-iteration load triggers.

SBUF pools are opened OUTSIDE the timing loop and the loop body is emitted
twice per hardware For iteration, so tile-pool buffer rotation carries across
iterations (cross-iteration load prefetch / store draining). Constants load
once, outside the loop.

All PE inputs are float16 (PSUM accumulation stays fp32): halves DMA traffic
(the bottleneck) vs f32, and PE transposes run 1.0 cycles/row vs 1.5 for f32r.
grid_chw is unused by the math (it enumerates the full (s, h) grid).
"""

import numpy as np

import concourse.bass as bass  # noqa: F401  (engine namespaces hang off nc)
import concourse.mybir as mybir
import concourse.tile as tile
from concourse import bacc
from concourse.bass_utils import run_bass_kernel_spmd

B, S, H, R = 4, 2048, 2048, 64
N_CORES = 8
P = 128
T = S // P            # 16 s-tiles (also 16 h-tiles)
OWN_T = T // 2        # 8 own s-tiles per core
CHUNK = 512
NCH = H // CHUNK      # 4 h-chunks
NG = 4                # s-tile groups of 4 (load granularity)

f16 = mybir.dt.float16
f32 = mybir.dt.float32

# Pair-wise AllReduce of the partial F_h: each core loads only its own seq
# half of hs (halves load DMA + einsum1) and the (b, j=0)/(b, j=1) pair
# reduces F_h over a device-to-device collective.
COLLECTIVE = False
CC_GROUPS = [[0, 1], [2, 3], [4, 5], [6, 7]]


def build_nc(reps: int = 1, mode: str = "full", unroll: bool = False):
    """Build + bacc-compile the SPMD kernel. reps>1 wraps two copies of the
    body in a hardware For loop of reps//2 (reps must be even), so tile-pool
    rotation double-buffers across iterations. Output is idempotent.

    mode (timing diagnostics, progressively enables phases):
      "dma"  : input loads + output stores only
      "e1"   : + einsum1 matmuls
      "tr"   : + PE transposes
      "e2"   : + einsum2 matmuls/copies
      "full" : + einsum3 (the real kernel)
    """
    lvl = {"dma": 0, "e1": 1, "tr": 2, "e2": 3, "full": 4}[mode]
    nc = bacc.Bacc(
        "TRN2",
        target_bir_lowering=False,
        debug=False,
        enable_asserts=True,
        num_devices=N_CORES,
    )

    # group g holds s-tiles 4g..4g+3 (of the rows this core receives);
    # per-partition line is 4*H fp16 = 16KB contiguous
    ngl = 2 if COLLECTIVE else NG       # groups loaded per core
    n_t = 4 * ngl                       # s-tiles contracted by einsum1 here
    hsg = nc.dram_tensor("hsg", [ngl, P, 4 * H], f16, kind="ExternalInput").ap()
    wst = nc.dram_tensor("wst", [P, n_t * R], f16, kind="ExternalInput").ap()
    wht = nc.dram_tensor("wht", [P, T * R], f16, kind="ExternalInput").ap()
    ident = nc.dram_tensor("ident", [P, P], f16, kind="ExternalInput").ap()
    # pair ip holds s-tiles (2ip, 2ip+1); 2*H fp16 = 8KB/partition contiguous
    out = nc.dram_tensor("out", [NG, P, 2 * H], f16, kind="ExternalOutput").ap()

    with tile.TileContext(nc) as tc:
        with (
            tc.tile_pool(name="const", bufs=1) as cpool,
            tc.tile_pool(name="raw", bufs=2 * ngl) as rpool,
            tc.tile_pool(name="hsT", bufs=8) as hpool,
            tc.tile_pool(name="fact", bufs=2) as fpool,
            tc.tile_pool(name="outsb", bufs=4) as opool,
            tc.tile_pool(name="ccb", bufs=2, space="DRAM") as ccpool,
        ):
            # constants: loaded once, outside the timing loop
            wst_t = cpool.tile([P, n_t * R], f16, tag="wst")
            wht_t = cpool.tile([P, T * R], f16, tag="wht")
            id_t = cpool.tile([P, P], f16, tag="id")
            nc.gpsimd.dma_start(wst_t[:], wst)
            nc.gpsimd.dma_start(wht_t[:], wht)
            nc.gpsimd.dma_start(id_t[:], ident)
            fake_out = None
            if lvl < 4:
                fake_out = cpool.tile([P, 2 * H], f16, tag="fake_out")
                nc.gpsimd.memset(fake_out[:], 0.0)

            def body(_=None):
                # per-group F_s.T tiles so einsum3's ip0/ip1 depend only on
                # group 0's copies (a single tile would serialize on group 1)
                fst_g = [
                    fpool.tile([P, 4 * P], f16, tag=f"fst{g}", name=f"fst{g}")
                    for g in range(2)
                ]
                fh_dup = fpool.tile([P, H], f16, tag="fh")            # [128, 2048]

                raws = []
                for g in range(ngl):
                    raw = rpool.tile([P, 4 * H], f16, tag="raw")
                    nc.sync.dma_start(raw[:], hsg[g])
                    raws.append(raw)

                with tc.tile_pool(name="ps_fh", bufs=1, space="PSUM") as fh_ps_pool:
                    # [128, 1024] split layout (2 PSUM banks instead of 4):
                    # chunks 0,1 live on partitions 0..63, chunks 2,3 on
                    # 64..127. Frees banks so einsum3's PSUM pool can coexist
                    # with the transpose pools (einsum3 interleaving below).
                    ps_fh = fh_ps_pool.tile([P, 2 * CHUNK], f32, tag="fh")

                    def fh_ps(c):
                        return ps_fh[(c // 2) * R:(c // 2 + 1) * R,
                                     (c % 2) * CHUNK:(c % 2 + 1) * CHUNK]

                    def e1(g):
                        # einsum1: accumulate partial F_h chunks from group g
                        raw = raws[g]
                        for q in range(4 if lvl >= 1 else 0):
                            t = 4 * g + q
                            for c in range(NCH):
                                nc.tensor.matmul(
                                    fh_ps(c),
                                    wst_t[:, t * R:(t + 1) * R],
                                    raw[:, q * H + c * CHUNK:q * H + (c + 1) * CHUNK],
                                    start=(t == 0),
                                    stop=(t == n_t - 1),
                                )

                    def tr_e2(g, tr_ps, fs_ps, unit_fn=None):
                        # einsum2 for own group g (s-tiles 4g..4g+3).
                        # einsum2's matmul for h-tile k is emitted after the
                        # transposes for k+1 (software pipelining), so the PE
                        # never stalls on the PSUM->SBUF copy of k. unit_fn
                        # (if given) interleaves one einsum3 unit per k.
                        raw = raws[g]
                        ps_fs_g = None
                        if lvl >= 3:
                            ps_fs_g = fs_ps.tile([R, CHUNK], f32, tag="fs")
                        hsTs = {}
                        for k in range(T + 1):  # h-tiles (+1 drain)
                            if k < T:
                                ps_tr = tr_ps.tile([P, 4 * P], f16, tag="tr")
                                for q in range(4):  # s-tiles 4g+q
                                    src = raw[:, q * H + k * P:q * H + (k + 1) * P]
                                    nc.tensor.matmul(
                                        ps_tr[:, q * P:(q + 1) * P],
                                        src,
                                        id_t[:],
                                        is_transpose=True,
                                        start=(q == 0),
                                        stop=(q == 3),
                                    )
                                hsT = hpool.tile([P, 4 * P], f16, tag="hsT")
                                if k % 3 == 1:
                                    nc.scalar.copy(hsT[:], ps_tr[:])
                                else:
                                    nc.vector.tensor_copy(hsT[:], ps_tr[:])
                                hsTs[k] = hsT
                            if lvl >= 3 and k > 0:
                                nc.tensor.matmul(
                                    ps_fs_g[:],
                                    wht_t[:, (k - 1) * R:k * R],
                                    hsTs.pop(k - 1),
                                    start=(k - 1 == 0),
                                    stop=(k - 1 == T - 1),
                                )
                            if unit_fn is not None and k >= 2:
                                unit_fn()
                        if lvl >= 3:
                            # F_s.T chunk -> SBUF, duplicated to both halves
                            nc.vector.tensor_copy(fst_g[g][0:R, :], ps_fs_g[:])
                            nc.scalar.copy(fst_g[g][R:2 * R, :], ps_fs_g[:])

                    def fh_copies():
                        # F_h -> SBUF, duplicated to both partition halves.
                        # Emitted before tr_e2(1): the copies run on DVE/Act
                        # while the PE does group 1's transposes, so einsum3
                        # starts with no PE idle (keeps the PE p-state high).
                        if lvl < 4:
                            return
                        # 4 copies of [64, 1024]; ph = psum partition half
                        # (fh cols ph*1024..): DVE serves dup-half 0, Act
                        # dup-half 1, lowest columns first so einsum3's
                        # earliest units unblock first.
                        for ph in range(2):
                            src = ps_fh[ph * R:(ph + 1) * R, :]
                            sl = slice(ph * 2 * CHUNK, (ph + 1) * 2 * CHUNK)
                            nc.vector.tensor_copy(fh_dup[0:R, sl], src)
                            nc.scalar.copy(fh_dup[R:2 * R, sl], src)

                    def fh_allreduce():
                        # partial F_h -> fp16 SBUF -> DRAM bounce -> pairwise
                        # AllReduce -> back into both halves of fh_dup. The
                        # whole chain overlaps the PE's transpose phase.
                        if lvl < 4:
                            return
                        fhp = fpool.tile([R, H], f16, tag="fhp")
                        for ph in range(2):
                            src = ps_fh[ph * R:(ph + 1) * R, :]
                            sl = slice(ph * 2 * CHUNK, (ph + 1) * 2 * CHUNK)
                            if ph == 0:
                                nc.vector.tensor_copy(fhp[:, sl], src)
                            else:
                                nc.scalar.copy(fhp[:, sl], src)
                        in_b = ccpool.tile([R, H], f16, tag="cc_in")
                        out_b = ccpool.tile([R, H], f16, tag="cc_out")
                        nc.scalar.dma_start(in_b[:], fhp[:])
                        nc.gpsimd.collective_compute(
                            "AllReduce",
                            mybir.AluOpType.add,
                            replica_groups=CC_GROUPS,
                            ins=[in_b.opt()],
                            outs=[out_b.opt()],
                        )
                        nc.gpsimd.dma_start(fh_dup[0:R, :], out_b[:])
                        nc.gpsimd.dma_start(fh_dup[R:2 * R, :], out_b[:])

                    # einsum3 (K=64, row-packed pairs) + output stores,
                    # emitted one [128, 512] unit at a time. ip0/ip1's 16
                    # units interleave into tr_e2(1)'s PE stream (they only
                    # need fst_g[0] + fh_dup, both ready by then), spreading
                    # the PSUM->SBUF cast copies across the transpose phase.
                    e3_state = {"n": 0, "outsb": None}

                    def e3_unit(po_pool):
                        if lvl < 4:
                            return
                        n = e3_state["n"]
                        if n >= 32:
                            return
                        e3_state["n"] = n + 1
                        ip, u = n // 8, n % 8
                        hf, c = u % 2, u // 2
                        i = 2 * ip + hf
                        base = R * hf
                        if u == 0:
                            e3_state["outsb"] = opool.tile(
                                [P, 2 * H], f16, tag="outsb", name="outsb"
                            )
                        outsb = e3_state["outsb"]
                        ps_o = po_pool.tile([P, CHUNK], f32, tag="o")
                        nc.tensor.matmul(
                            ps_o[:],
                            fst_g[i // 4][base:base + R, (i % 4) * P:(i % 4 + 1) * P],
                            fh_dup[base:base + R, c * CHUNK:(c + 1) * CHUNK],
                            start=True,
                            stop=True,
                        )
                        dst = outsb[:, hf * H + c * CHUNK:(hf * H + (c + 1) * CHUNK)]
                        if n % 2 == 0:
                            nc.vector.tensor_copy(dst, ps_o[:])
                        else:
                            nc.scalar.copy(dst, ps_o[:])
                        if u == 7:
                            nc.gpsimd.dma_start(out[ip], outsb[:])

                    with (
                        tc.tile_pool(name="ps_tr", bufs=2, space="PSUM") as tr_ps,
                        tc.tile_pool(name="ps_fs", bufs=2, space="PSUM") as fs_ps,
                        tc.tile_pool(name="ps_oa", bufs=2, space="PSUM") as po_a,
                    ):
                        if COLLECTIVE:
                            e1(0)
                            e1(1)
                            fh_allreduce()
                            if lvl >= 2:
                                tr_e2(0, tr_ps, fs_ps)
                                tr_e2(1, tr_ps, fs_ps,
                                      unit_fn=lambda: e3_unit(po_a))
                        else:
                            e1(0)
                            if lvl >= 2:
                                tr_e2(0, tr_ps, fs_ps)
                            e1(1)
                            e1(2)
                            e1(3)
                            fh_copies()
                            if lvl >= 2:
                                tr_e2(1, tr_ps, fs_ps,
                                      unit_fn=lambda: e3_unit(po_a))
                        # finish ip0/ip1 (15 slots in the loop, 16 units)
                        while lvl >= 4 and e3_state["n"] < 16:
                            e3_unit(po_a)

                # remaining einsum3 units (ip2/ip3) with a deep PSUM pool so
                # the PE runs far ahead of the cast copies at the tail
                with tc.tile_pool(name="ps_o", bufs=6, space="PSUM") as po_b:
                    if lvl < 4:
                        for ip in range(OWN_T // 2):
                            nc.gpsimd.dma_start(out[ip], fake_out[:])
                    else:
                        while e3_state["n"] < 32:
                            e3_unit(po_b)

            if unroll:
                for _ in range(reps):
                    body()
            elif reps == 1:
                body()
            else:
                nbody = 4 if reps % 4 == 0 else 2
                assert reps % nbody == 0, "reps must be even (2 bodies per For_i)"
                with tc.For_i(0, reps // nbody, 1):
                    for _ in range(nbody):
                        body()

    nc.compile()
    return nc


def _tile_weight(w_t: np.ndarray) -> np.ndarray:
    """[2048, 64] -> [128, 16*64] stack where tile t = cols [64t:64t+64]."""
    return np.ascontiguousarray(
        w_t.reshape(T, P, R).transpose(1, 0, 2).reshape(P, T * R)
    ).astype(np.float16)


def _pack_hs(hs_c: np.ndarray) -> np.ndarray:
    """[rows, 2048] -> [rows/512, 128, 4*H]: group g = s-tiles 4g..4g+3, row p
    of group g holds tiles' rows s = g*512 + q*128 + p concatenated over q."""
    ng = hs_c.shape[0] // 512
    return np.ascontiguousarray(
        hs_c.reshape(ng, 4, P, H).transpose(0, 2, 1, 3).reshape(ng, P, 4 * H)
    )


def _unpack_out(o: np.ndarray) -> np.ndarray:
    """[NG, 128, 2*H] -> [1024, 2048]: s = ip*256 + w*128 + p."""
    return np.ascontiguousarray(
        o.reshape(NG, P, 2, H).transpose(0, 2, 1, 3).reshape(S // 2, H)
    )


def _tile_weight_half(w_t: np.ndarray) -> np.ndarray:
    """[1024, 64] -> [128, 8*64] stack where tile t = cols [64t:64t+64]."""
    return np.ascontiguousarray(
        w_t.reshape(T // 2, P, R).transpose(1, 0, 2).reshape(P, T // 2 * R)
    ).astype(np.float16)


def build_in_maps(hs_all: np.ndarray, w_seq: np.ndarray, w_hid: np.ndarray):
    ident = np.eye(P, dtype=np.float16)
    wht_tiled = _tile_weight(np.ascontiguousarray(w_hid.T))
    hs_f16 = hs_all.astype(np.float16)
    if COLLECTIVE:
        wst_halves = {
            j: _tile_weight_half(
                np.ascontiguousarray(w_seq.T[j * (S // 2):(j + 1) * (S // 2)])
            )
            for j in range(2)
        }
    else:
        wst_halves = {
            j: _tile_weight(np.roll(w_seq.T, -(S // 2) * j, axis=0))
            for j in range(2)
        }
    in_maps = []
    for c in range(N_CORES):
        b, j = c // 2, c % 2
        hsb = hs_f16[b]
        if COLLECTIVE:
            hs_c = hsb[j * (S // 2):(j + 1) * (S // 2)]
        else:
            hs_c = hsb if j == 0 else np.roll(hsb, -(S // 2), axis=0)
        in_maps.append(
            {"hsg": _pack_hs(hs_c), "wst": wst_halves[j], "wht": wht_tiled,
             "ident": ident}
        )
    return in_maps


_NC_CACHE: dict = {}


def kernel(**inputs) -> np.ndarray:
    hs_all = np.asarray(inputs["hidden_states"], dtype=np.float32)
    w_seq = np.asarray(inputs["W_seq"], dtype=np.float32)
    w_hid = np.asarray(inputs["W_hid"], dtype=np.float32)

    if "nc" not in _NC_CACHE:
        _NC_CACHE["nc"] = build_nc(1)
    nc = _NC_CACHE["nc"]

    in_maps = build_in_maps(hs_all, w_seq, w_hid)
    res = run_bass_kernel_spmd(nc, in_maps, core_ids=list(range(N_CORES)))

    out_full = np.empty((B, S, H), dtype=np.float32)
    for c in range(N_CORES):
        b, j = c // 2, c % 2
        out_full[b, j * (S // 2):(j + 1) * (S // 2), :] = _unpack_out(
            res.results[c]["out"]
        )
    return out_full
